# revision 1
# baseline (speedup 1.0000x reference)
"""Trainium2 Bass kernel: multi-head attention (B=2, S=2048, E=1024, H=16).

Sharding: 8 cores = 2 batches x 4 head-groups. Core c handles batch c//4 and
heads [4*(c%4), 4*(c%4)+4) (256 feature columns of the projections).

Per-core device program (all matmuls in fp32r):
  - inputs: xT [E,S] (host-transposed x[b]), wqT/wkT/wvT [E,256] (host-
    transposed row-slices of Wq/Wk/Wv), woT [256,E] (host-transposed column
    slice of Wo).
  - qT,kT [256,S] = (x @ W^T)^T per head-group, computed directly in [f,s]
    layout; v [S,256] in [s,f] layout with a ones column appended per head.
  - per (head, qi-chunk): scores^T tiles [128 kj, 512 qi] on PE, exp on ACT
    (sm_scale folded into the activation scale), attn@v accumulated on PE with
    the ones column producing the softmax denominator in partition 64,
    then reciprocal + GPSIMD partition-broadcast + multiply to normalize;
    output kept in [f, s] layout for the output projection.
  - out_partial [S,E] = o^T^T @ Wo^T column-slice; host sums 4 partials per
    batch and adds bo.
"""

import numpy as np

import concourse.tile as tile
import concourse.mybir as mybir
from concourse import bacc
from concourse.bass_utils import run_bass_kernel_spmd

B, S, E, H, D = 2, 2048, 1024, 16, 64
NCORES = 8
GPB = NCORES // B      # head-groups (cores) per batch = 4
HPC = H // GPB         # heads per core = 4
FPC = HPC * D          # feature cols per core = 256
SM = float(D) ** -0.5  # softmax scale

F32 = mybir.dt.float32
F32R = mybir.dt.float32r

P = 128
NE = E // P            # 8 e-tiles
NST = S // P           # 16 s-tiles (key tiles)
NQ = 4                 # qi chunks
QC = S // NQ           # 512
KTG = 2                # k-tiles per psum/exp group
NKG = NST // KTG       # 8 groups
FT = FPC // P          # 2 f-tiles per core


def _round_fp32r(a: np.ndarray) -> np.ndarray:
    """Round fp32 to the fp32r encoding (RNE to 12-bit mantissa)."""
    u = np.ascontiguousarray(a, dtype=np.float32).view(np.uint32)
    lo = u & np.uint32(0xFFF)
    base = u & ~np.uint32(0xFFF)
    rup = (lo > 0x800) | ((lo == 0x800) & (((base >> np.uint32(12)) & np.uint32(1)) == 1))
    out = base + (rup.astype(np.uint32) << np.uint32(12))
    return out.view(np.float32)


def _build():
    nc = bacc.Bacc("TRN2", target_bir_lowering=False, debug=False)

    xT_d = nc.dram_tensor("xT", [E, S], F32R, kind="ExternalInput")
    wq_d = nc.dram_tensor("wqT", [E, FPC], F32R, kind="ExternalInput")
    wk_d = nc.dram_tensor("wkT", [E, FPC], F32R, kind="ExternalInput")
    wv_d = nc.dram_tensor("wvT", [E, FPC], F32R, kind="ExternalInput")
    wo_d = nc.dram_tensor("woT", [FPC, E], F32R, kind="ExternalInput")
    ones_lhs_d = nc.dram_tensor("ones_lhs", [1, D], F32R, kind="ExternalInput")
    ones_col_d = nc.dram_tensor("ones_col", [P, HPC, 1], F32R, kind="ExternalInput")
    out_d = nc.dram_tensor("out", [S, E], F32, kind="ExternalOutput")

    with tile.TileContext(nc) as tc:
        with (
            tc.tile_pool(name="wpool", bufs=1) as wpool,
            tc.tile_pool(name="xpool", bufs=1) as xpool,
            tc.tile_pool(name="qkpool", bufs=1) as qkpool,
            tc.tile_pool(name="vpool", bufs=1) as vpool,
            tc.tile_pool(name="opool", bufs=1) as opool,
            tc.tile_pool(name="epool", bufs=3) as epool,
            tc.tile_pool(name="spool", bufs=2) as spool,
            tc.tile_pool(name="outpool", bufs=3) as outpool,
            tc.tile_pool(name="pspool", bufs=2, space="PSUM") as pspool,
            tc.tile_pool(name="popool", bufs=2, space="PSUM") as popool,
            tc.tile_pool(name="oaccpool", bufs=2, space="PSUM") as oaccpool,
        ):
            # ---- weights / constants -------------------------------------
            wq = wpool.tile([P, NE, FPC], F32R, name="wq")
            wk = wpool.tile([P, NE, FPC], F32R, name="wk")
            wv = wpool.tile([P, NE, FPC], F32R, name="wv")
            wo = wpool.tile([P, FT, E], F32R, name="wo")
            ones = wpool.tile([1, D], F32R, name="ones")
            wk_r = wk_d.ap().rearrange("(t p) f -> p t f", p=P)
            wq_r = wq_d.ap().rearrange("(t p) f -> p t f", p=P)
            # f-tile-0 halves first: only they gate the first score matmuls;
            # the ft1 halves ride behind the early x chunks.
            nc.sync.dma_start(out=wk[:, :, 0:P], in_=wk_r[:, :, 0:P])
            nc.sync.dma_start(out=wq[:, :, 0:P], in_=wq_r[:, :, 0:P])

            # ---- x^T (chunk-major DMA so compute starts early) -----------
            xT_r = xT_d.ap().rearrange("(t p) s -> p t s", p=P)
            xts = [
                xpool.tile([P, S], F32R, name=f"xt{et}", tag=f"xt{et}")
                for et in range(NE)
            ]
            for cq in range(NQ):
                csl = slice(cq * QC, (cq + 1) * QC)
                for et in range(NE):
                    nc.sync.dma_start(out=xts[et][:, csl], in_=xT_r[:, et, csl])
                if cq == 0:
                    nc.sync.dma_start(
                        out=wv, in_=wv_d.ap().rearrange("(t p) f -> p t f", p=P)
                    )
                    nc.sync.dma_start(out=ones, in_=ones_lhs_d.ap())
                elif cq == 2:
                    nc.sync.dma_start(out=wk[:, :, P:FPC], in_=wk_r[:, :, P:FPC])
                    nc.sync.dma_start(out=wq[:, :, P:FPC], in_=wq_r[:, :, P:FPC])

            nc.sync.dma_start(out=wo, in_=wo_d.ap().rearrange("(t p) g -> p t g", p=P))

            # ---- v projection: v[s, f] with ones col per head ------------
            v_tiles = [
                vpool.tile([P, HPC, D + 1], F32R, name=f"v{st}", tag=f"v{st}")
                for st in range(NST)
            ]

            def proj_v(st):
                vt = v_tiles[st]
                nc.sync.dma_start(out=vt[:, :, D : D + 1], in_=ones_col_d.ap())
                ps_v = popool.tile([P, FPC], F32, name="ps_v", tag="po")
                for et in range(NE):
                    nc.tensor.matmul(
                        ps_v,
                        xts[et][:, st * P : (st + 1) * P],
                        wv[:, et, :],
                        start=(et == 0),
                        stop=(et == NE - 1),
                    )
                nc.vector.tensor_copy(
                    vt[:, :, 0:D], ps_v.rearrange("p (h d) -> p h d", d=D)
                )

            # ---- q^T / k^T projections: [f, s] ---------------------------
            def proj_T(w_tile, dst_tiles, which, ft, cq):
                ps = popool.tile([P, QC], F32, name=f"ps_{which}", tag="po")
                for et in range(NE):
                    nc.tensor.matmul(
                        ps,
                        w_tile[:, et, ft * P : (ft + 1) * P],
                        xts[et][:, cq * QC : (cq + 1) * QC],
                        start=(et == 0),
                        stop=(et == NE - 1),
                    )
                nc.vector.tensor_copy(
                    dst_tiles[ft][:, cq * QC : (cq + 1) * QC], ps
                )

            kts = [qkpool.tile([P, S], F32R, name=f"kt{ft}", tag=f"kt{ft}") for ft in range(FT)]
            qts = [qkpool.tile([P, S], F32R, name=f"qt{ft}", tag=f"qt{ft}") for ft in range(FT)]
            ots = [opool.tile([P, S], F32R, name=f"ot{ft}", tag=f"ot{ft}") for ft in range(FT)]

            # Filler machinery: generators that emit one PE-side instruction
            # per next() call. attn_core drains a couple of units after each
            # kt step, so independent matmul work lands inside the PE idle
            # gaps of the ACT-bound attention inner loop instead of between
            # cores (the PE executes its stream in order).
            from collections import deque

            fillers = deque()

            def pump(n):
                for _ in range(n):
                    while fillers:
                        try:
                            next(fillers[0])
                            break
                        except StopIteration:
                            fillers.popleft()
                    else:
                        return

            def attn_core(pair, cq, per_kt=2):
                """Heads 2*pair, 2*pair+1 for query chunk cq; the two heads'
                score matmuls run concurrently on PE row-groups 0-63/64-127.
                Returns the two accumulation psum tiles (rows 0..63 =
                sum(exp*v), row 64 = sum(exp))."""
                ft = pair
                csl = slice(cq * QC, (cq + 1) * QC)
                ps_o = [
                    oaccpool.tile([D + 1, QC], F32, name=f"ps_o{s}", tag="oacc")
                    for s in range(2)
                ]
                for kt in range(NST):
                    et_t = epool.tile([P, 2, QC], F32R, name="et_t", tag="et_t")
                    ps_s = pspool.tile([P, 2, QC], F32, name="ps_s", tag="ps_s")
                    for sub in range(2):
                        lo, hi = sub * D, (sub + 1) * D
                        nc.tensor.matmul(
                            ps_s[:, sub, :],
                            kts[ft][lo:hi, kt * P : (kt + 1) * P],
                            qts[ft][lo:hi, csl],
                            start=True,
                            stop=True,
                        )
                    nc.scalar.activation(
                        out=et_t,
                        in_=ps_s,
                        func=mybir.ActivationFunctionType.Exp,
                        scale=SM,
                    )
                    for sub in range(2):
                        nc.tensor.matmul(
                            ps_o[sub],
                            v_tiles[kt][:, 2 * pair + sub, :],
                            et_t[:, sub, :],
                            start=(kt == 0),
                            stop=(kt == NST - 1),
                        )
                    if kt > 0:
                        pump(per_kt)
                return ps_o

            def attn_drain(ps_o):
                """Copy both accumulators (incl. the sum row) to SBUF right
                away so the psum slots free early."""
                o_full = []
                for sub in range(2):
                    of = epool.tile([D + 1, QC], F32, name="o_hat", tag="o_hat", bufs=4)
                    nc.vector.tensor_copy(of, ps_o[sub])
                    o_full.append(of)
                return o_full

            def bcast_recip(o_full):
                """Reciprocal of each sum row, partition-broadcast on the
                (otherwise idle) GPSIMD engine. No PE/ACT work."""
                bcs = []
                for sub in range(2):
                    rec = spool.tile([1, QC], F32, name="rec", tag="rec", bufs=1)
                    nc.vector.reciprocal(rec, o_full[sub][D : D + 1, :])
                    bc = spool.tile([D, QC], F32, name="bc", tag="bc", bufs=4)
                    nc.gpsimd.partition_broadcast(bc, rec)
                    bcs.append(bc)
                return bcs

            def attn_finish(pair, cq, o_full):
                """Normalize a pair-0 chunk (full-width multiply)."""
                csl = slice(cq * QC, (cq + 1) * QC)
                bcs = bcast_recip(o_full)
                for sub in range(2):
                    lo, hi = sub * D, (sub + 1) * D
                    nc.vector.tensor_mul(
                        ots[pair][lo:hi, csl], o_full[sub][0:D, :], bcs[sub]
                    )

            def finish_outproj_units(cq, o_full, bcs, tail=False):
                """Pair-1 normalize pipelined with the output projection at
                s-tile granularity (shortens the kernel tail). In the tail
                the PSUM->SBUF copies ride the idle ACT engine instead of
                DVE."""
                for sti in range(NQ):
                    st = cq * NQ + sti
                    ssl = slice(sti * P, (sti + 1) * P)
                    for sub in range(2):
                        lo, hi = sub * D, (sub + 1) * D
                        nc.vector.tensor_mul(
                            ots[1][lo:hi, st * P : (st + 1) * P],
                            o_full[sub][0:D, ssl],
                            bcs[sub][:, ssl],
                        )
                    yield
                    out_sb = outpool.tile([P, E], F32, name="out_sb", tag="out_sb")
                    for gc in range(2):
                        ps_out = popool.tile([P, QC], F32, name="ps_out", tag="po")
                        for ft in range(FT):
                            nc.tensor.matmul(
                                ps_out,
                                ots[ft][:, st * P : (st + 1) * P],
                                wo[:, ft, gc * QC : (gc + 1) * QC],
                                start=(ft == 0),
                                stop=(ft == FT - 1),
                            )
                            yield
                        if tail:
                            nc.scalar.activation(
                                out=out_sb[:, gc * QC : (gc + 1) * QC],
                                in_=ps_out,
                                func=mybir.ActivationFunctionType.Copy,
                            )
                        else:
                            nc.vector.tensor_copy(
                                out_sb[:, gc * QC : (gc + 1) * QC], ps_out
                            )
                        yield
                    nc.sync.dma_start(
                        out=out_d.ap()[st * P : (st + 1) * P, :], in_=out_sb
                    )

            # Emission order = scheduler priority. Attention cores are
            # emitted right after the projections of their own chunk, so the
            # first exp fires as soon as chunk-0 data exists; later-chunk
            # projections backfill PE whenever attention is dep-blocked.
            def proj1_units():
                for cq in range(NQ):
                    for w_tile, dst, which in ((wk, kts, "k1"), (wq, qts, "q1")):
                        ps = popool.tile([P, QC], F32, name=f"ps_{which}", tag="po")
                        for et in range(NE):
                            nc.tensor.matmul(
                                ps,
                                w_tile[:, et, P : 2 * P],
                                xts[et][:, cq * QC : (cq + 1) * QC],
                                start=(et == 0),
                                stop=(et == NE - 1),
                            )
                            yield
                        nc.vector.tensor_copy(
                            dst[1][:, cq * QC : (cq + 1) * QC], ps
                        )
                        yield

            for cq in range(NQ):
                proj_T(wk, kts, "k0", 0, cq)
                proj_T(wq, qts, "q0", 0, cq)
                for st in range(cq * NQ, (cq + 1) * NQ):
                    proj_v(st)

            PER_KT = {(0, 1): 2}
            for pair in range(2):
                for cq in range(NQ):
                    ps_o = attn_core(pair, cq, per_kt=PER_KT.get((pair, cq), 2 if pair else 1))
                    of = attn_drain(ps_o)
                    if pair == 0:
                        attn_finish(pair, cq, of)
                    elif cq < NQ - 1:
                        bcs = bcast_recip(of)
                        fillers.append(finish_outproj_units(cq, of, bcs))
                    else:
                        # tail chunk: broadcast via a PE matmul (shortest
                        # latency chain right after the last core)
                        bcs = []
                        for sub in range(2):
                            rec = spool.tile([1, QC], F32, name="rec", tag="rec", bufs=1)
                            nc.vector.reciprocal(rec, of[sub][D : D + 1, :])
                            rec_r = spool.tile([1, QC], F32R, name="rec_r", tag="rec_r", bufs=1)
                            nc.vector.tensor_copy(rec_r, rec)
                            ps_bc = popool.tile([D, QC], F32, name="ps_bc", tag="po")
                            nc.tensor.matmul(ps_bc, ones, rec_r, start=True, stop=True)
                            bcs.append(ps_bc)
                        fillers.appendleft(
                            finish_outproj_units(cq, of, bcs, tail=True)
                        )
                    if pair == 0 and cq == 0:
                        fillers.append(proj1_units())
            # drain remaining fillers (the last chunk's output projection)
            while fillers:
                pump(64)

    nc.compile()
    return nc


_NC_CACHE = None


def _get_nc():
    global _NC_CACHE
    if _NC_CACHE is None:
        _NC_CACHE = _build()
    return _NC_CACHE


def make_in_maps(x, Wq, Wk, Wv, Wo):
    in_maps = []
    xTs = [_round_fp32r(x[b].T) for b in range(B)]
    for c in range(NCORES):
        b, hg = c // GPB, c % GPB
        fsl = slice(hg * FPC, (hg + 1) * FPC)
        in_maps.append({
            "xT": xTs[b],
            "wqT": _round_fp32r(Wq[fsl, :].T),
            "wkT": _round_fp32r(Wk[fsl, :].T),
            "wvT": _round_fp32r(Wv[fsl, :].T),
            "woT": _round_fp32r(Wo[:, fsl].T),
            "ones_lhs": np.ones((1, D), dtype=np.float32),
            "ones_col": np.ones((P, HPC, 1), dtype=np.float32),
        })
    return in_maps


def kernel(x, Wq, bq, Wk, bk, Wv, bv, Wo, bo):
    x = np.asarray(x, dtype=np.float32)
    Wq, Wk, Wv, Wo = (np.asarray(a, dtype=np.float32) for a in (Wq, Wk, Wv, Wo))
    bq, bk, bv, bo = (np.asarray(a, dtype=np.float32) for a in (bq, bk, bv, bo))
    if np.any(bq) or np.any(bk) or np.any(bv):
        # fall back: fold nonzero projection biases into an augmented input
        # row is not implemented; biases are zero for this problem spec.
        raise NotImplementedError("nonzero projection biases not supported")

    nc = _get_nc()
    in_maps = make_in_maps(x, Wq, Wk, Wv, Wo)
    res = run_bass_kernel_spmd(nc, in_maps, core_ids=list(range(NCORES)))
    out = np.empty((B, S, E), dtype=np.float32)
    for b in range(B):
        acc = res.results[b * GPB]["out"].astype(np.float32).copy()
        for hg in range(1, GPB):
            acc += res.results[b * GPB + hg]["out"]
        out[b] = acc
    out += bo[None, None, :]
    return out



# revision 9
# speedup vs baseline: 1.1016x; 1.1016x over previous
"""Trainium2 Bass kernel: multi-head attention (B=2, S=2048, E=1024, H=16).

Sharding: 8 cores = 2 batches x 4 head-groups. Core c handles batch c//4 and
heads [4*(c%4), 4*(c%4)+4) (256 feature columns of the projections).

v2 design (all-bf16, transposed attn@v):
  - inputs in bf16: xT [E,S], wqT/wkT/wvT [E,256], woT [256,E].
  - qT,kT [256,S] bf16 in [f,s] layout; v [S,256] bf16 in [s,f] layout with a
    ones column per head (col 64) that produces the softmax denominator.
  - scores^T tiles [128 kj, 512 qi] on PE (2 heads per exp tile), exp on ACT
    (sm_scale folded into the activation scale) -> et bf16.
  - attn@v TRANSPOSED: out [128 qi, 65] = et[kj,qi]^T-contract v[kj,65]; the
    65th column accumulates the denominator. Normalize fuses into the PSUM
    drain as a per-partition tensor_scalar multiply by 1/denom.
  - o_sb [128 qi, 128 f(2 heads)] is transposed to oT [f, s] via the DMA XBAR
    transpose (16x128 tiles), then out = oT^T @ wo per s-tile, out bf16 DMA.
  - host sums 4 partials per batch and adds bo.
"""

import numpy as np
import ml_dtypes

from collections import deque

import concourse.tile as tile
import concourse.mybir as mybir
from concourse import bacc
from concourse.bass_utils import run_bass_kernel_spmd

B, S, E, H, D = 2, 2048, 1024, 16, 64
NCORES = 8
GPB = NCORES // B      # head-groups (cores) per batch = 4
HPC = H // GPB         # heads per core = 4
FPC = HPC * D          # feature cols per core = 256
SM = float(D) ** -0.5  # softmax scale

BF16 = mybir.dt.bfloat16
F32 = mybir.dt.float32

P = 128
NE = E // P            # 8 e-tiles
NST = S // P           # 16 s-tiles (key tiles)
NQ = 4                 # qi chunks
QC = S // NQ           # 512
NQT = QC // P          # 4 qi-tiles per chunk
FT = FPC // P          # 2 f-tiles (head pairs) per core


def _build():
    nc = bacc.Bacc("TRN2", target_bir_lowering=False, debug=False)

    xT_d = nc.dram_tensor("xT", [E, S], BF16, kind="ExternalInput")
    wq_d = nc.dram_tensor("wqT", [E, FPC], BF16, kind="ExternalInput")
    wk_d = nc.dram_tensor("wkT", [E, FPC], BF16, kind="ExternalInput")
    wv_d = nc.dram_tensor("wvT", [E, FPC], BF16, kind="ExternalInput")
    wo_d = nc.dram_tensor("woT", [FPC, E], BF16, kind="ExternalInput")
    out_d = nc.dram_tensor("out", [S, E], BF16, kind="ExternalOutput")

    with tile.TileContext(nc) as tc:
        with (
            tc.tile_pool(name="wpool", bufs=1) as wpool,
            tc.tile_pool(name="xpool", bufs=1) as xpool,
            tc.tile_pool(name="qkpool", bufs=1) as qkpool,
            tc.tile_pool(name="vpool", bufs=1) as vpool,
            tc.tile_pool(name="opool", bufs=1) as opool,
            tc.tile_pool(name="epool", bufs=3) as epool,
            tc.tile_pool(name="spool", bufs=2) as spool,
            tc.tile_pool(name="outpool", bufs=2) as outpool,
            tc.tile_pool(name="pspool", bufs=2, space="PSUM") as pspool,
            tc.tile_pool(name="popool", bufs=2, space="PSUM") as popool,
            tc.tile_pool(name="oaccpool", bufs=1, space="PSUM") as oaccpool,
        ):
            # ---- weights / x DMA (emission order = DMA queue order) -------
            wq = wpool.tile([P, NE, FPC], BF16, name="wq")
            wk = wpool.tile([P, NE, FPC], BF16, name="wk")
            wv = wpool.tile([P, NE, FPC], BF16, name="wv")
            wo = wpool.tile([P, FT, E], BF16, name="wo")
            wk_r = wk_d.ap().rearrange("(t p) f -> p t f", p=P)
            wq_r = wq_d.ap().rearrange("(t p) f -> p t f", p=P)
            nc.sync.dma_start(out=wk, in_=wk_r)

            xT_r = xT_d.ap().rearrange("(t p) s -> p t s", p=P)
            xts = [
                xpool.tile([P, S], BF16, name=f"xt{et}", tag=f"xt{et}")
                for et in range(NE)
            ]
            # chunk 0: per-e-tile DMAs so projection matmuls stream behind
            for et in range(4):
                nc.sync.dma_start(out=xts[et][:, 0:QC], in_=xT_r[:, et, 0:QC])
            nc.sync.dma_start(out=wq, in_=wq_r)
            for et in range(4, NE):
                nc.sync.dma_start(out=xts[et][:, 0:QC], in_=xT_r[:, et, 0:QC])
            nc.sync.dma_start(
                out=wv, in_=wv_d.ap().rearrange("(t p) f -> p t f", p=P)
            )
            # chunks 1-3: one DMA per (chunk, e-tile) to keep deps tile-level
            for cq in range(1, NQ):
                csl = slice(cq * QC, (cq + 1) * QC)
                for et in range(NE):
                    nc.sync.dma_start(out=xts[et][:, csl], in_=xT_r[:, et, csl])
            nc.sync.dma_start(out=wo, in_=wo_d.ap().rearrange("(t p) g -> p t g", p=P))

            # ---- SBUF working tiles --------------------------------------
            kts = [qkpool.tile([P, S], BF16, name=f"kt{ft}", tag=f"kt{ft}") for ft in range(FT)]
            qts = [qkpool.tile([P, S], BF16, name=f"qt{ft}", tag=f"qt{ft}") for ft in range(FT)]
            oTs = [opool.tile([P, S], BF16, name=f"oT{ft}", tag=f"oT{ft}") for ft in range(FT)]
            v_tiles = [
                vpool.tile([P, HPC, D + 1], BF16, name=f"v{st}", tag=f"v{st}")
                for st in range(NST)
            ]
            # ones column for the denominators (gpsimd memset, no DMA)
            for st in range(NST):
                nc.gpsimd.memset(v_tiles[st][:, :, D : D + 1], 1.0)

            # ---- filler machinery: generators emit one PE-side instruction
            # per next() call so independent matmul work lands inside the PE
            # idle gaps of the ACT-bound attention loop. Units can be force-
            # finished to guarantee writer-before-reader EMISSION order (the
            # tile framework only tracks deps on already-emitted writers).
            fillers = deque()

            class Unit:
                def __init__(self, gen):
                    self.gen = gen
                    self.done = False

                def step(self):
                    if self.done:
                        return False
                    try:
                        next(self.gen)
                        return True
                    except StopIteration:
                        self.done = True
                        return False

                def finish(self):
                    while not self.done:
                        self.step()
                    if fillers and fillers[0] is self:
                        fillers.popleft()

            def add_filler(gen, front=False):
                u = Unit(gen)
                if front:
                    fillers.appendleft(u)
                else:
                    fillers.append(u)
                return u

            def pump(n):
                for _ in range(n):
                    while fillers:
                        if fillers[0].step():
                            break
                        fillers.popleft()
                    else:
                        return

            # ---- projection groups ---------------------------------------
            def proj_qk_units(w_tile, dst, ft, cq):
                ps = popool.tile([P, QC], F32, name="ps_qk", tag="po")
                csl = slice(cq * QC, (cq + 1) * QC)
                for et in range(NE):
                    nc.tensor.matmul(
                        ps,
                        w_tile[:, et, ft * P : (ft + 1) * P],
                        xts[et][:, csl],
                        start=(et == 0),
                        stop=(et == NE - 1),
                    )
                    yield
                nc.vector.tensor_copy(dst[ft][:, csl], ps)
                yield

            def proj_v_units(st):
                vt = v_tiles[st]
                ps_v = popool.tile([P, FPC], F32, name="ps_v", tag="po")
                for et in range(NE):
                    nc.tensor.matmul(
                        ps_v,
                        xts[et][:, st * P : (st + 1) * P],
                        wv[:, et, :],
                        start=(et == 0),
                        stop=(et == NE - 1),
                    )
                    yield
                nc.vector.tensor_copy(
                    vt[:, :, 0:D], ps_v.rearrange("p (h d) -> p h d", d=D)
                )
                yield

            def run_units(gen):
                for _ in gen:
                    pass

            # ---- output projection (per s-tile, pipelined as filler) -----
            def outproj_units(st):
                out_sb = outpool.tile([P, E], BF16, name="out_sb", tag="out_sb")
                for gc in range(2):
                    ps_out = popool.tile([P, QC], F32, name="ps_out", tag="po")
                    for ft in range(FT):
                        nc.tensor.matmul(
                            ps_out,
                            oTs[ft][:, st * P : (st + 1) * P],
                            wo[:, ft, gc * QC : (gc + 1) * QC],
                            start=(ft == 0),
                            stop=(ft == FT - 1),
                        )
                        yield
                    nc.vector.tensor_copy(out_sb[:, gc * QC : (gc + 1) * QC], ps_out)
                    yield
                nc.sync.dma_start(
                    out=out_d.ap()[st * P : (st + 1) * P, :], in_=out_sb
                )

            # ---- attention -----------------------------------------------
            # Phase 1 per (pair, cq): 16 kt-steps of scores+exp; the 16 exp
            # tiles are retained in SBUF (double-buffered across chunks).
            # Phase 2 (pumped as filler during the NEXT chunk's phase 1):
            # per acc (qt, sub), 16 contiguous attn@v matmuls into one of two
            # ping-pong PSUM banks (HW allows only one open accumulation
            # group per bank), then recip+normalize drain and, per qt, an
            # XBAR transpose into oT.
            oaccs = [
                oaccpool.tile([P, D + 1], F32, name=f"oacc{i}", tag=f"oacc{i}")
                for i in range(2)
            ]

            gi = [0]  # global attn@v group parity for psum ping-pong

            def attnv_group(pair, cq, ets, qt, sub, o_sb):
                """One contiguous attn@v accumulation group (16 matmuls) plus
                its reciprocal + normalize drain. HW allows only one open
                accumulation group per PSUM bank, hence contiguous + two
                ping-pong banks."""
                acc = oaccs[gi[0] % 2]
                gi[0] += 1
                for kt in range(NST):
                    nc.tensor.matmul(
                        acc,
                        ets[kt][:, sub, qt * P : (qt + 1) * P],
                        v_tiles[kt][:, 2 * pair + sub, :],
                        start=(kt == 0),
                        stop=(kt == NST - 1),
                    )
                r = spool.tile([P, 1], F32, name="r", tag="r", bufs=4)
                nc.vector.reciprocal(r, acc[:, D : D + 1])
                nc.vector.tensor_scalar(
                    out=o_sb[:, sub, :],
                    in0=acc[:, 0:D],
                    scalar1=r,
                    scalar2=None,
                    op0=mybir.AluOpType.mult,
                )

            def attnv_finish_qt(pair, cq, qt, o_sb):
                nc.sync.dma_start_transpose(
                    out=oTs[pair][:, cq * QC + qt * P : cq * QC + (qt + 1) * P],
                    in_=o_sb,
                )
                if pair == 1:
                    add_filler(outproj_units(cq * NQT + qt))

            def attn_phase1(pair, cq, per_kt, ets, prev=None, kt_gate=None):
                """Scores+exp for (pair, cq); the PREVIOUS chunk's attn@v
                groups are interleaved deterministically, one per two
                kt-steps, so every engine stream is emitted in a feasible
                execution order."""
                csl = slice(cq * QC, (cq + 1) * QC)
                o_sb = [None]
                for kt in range(NST):
                    if kt_gate is not None and kt_gate(kt) is not None:
                        kt_gate(kt).finish()
                    ps_s = pspool.tile([P, 2, QC], F32, name="ps_s", tag="ps_s")
                    et_t = epool.tile([P, 2, QC], BF16, name=f"et{kt}", tag=f"et{kt}", bufs=3)
                    ets.append(et_t)
                    for sub in range(2):
                        lo, hi = sub * D, (sub + 1) * D
                        nc.tensor.matmul(
                            ps_s[:, sub, :],
                            kts[pair][lo:hi, kt * P : (kt + 1) * P],
                            qts[pair][lo:hi, csl],
                            start=True,
                            stop=True,
                        )
                    nc.scalar.activation(
                        out=et_t,
                        in_=ps_s,
                        func=mybir.ActivationFunctionType.Exp,
                        scale=SM,
                    )
                    if prev is not None and kt % 2 == 1:
                        g = kt // 2
                        qt, sub = g // 2, g % 2
                        if sub == 0:
                            o_sb[0] = epool.tile(
                                [P, 2, D], BF16, name="o_sb", tag="o_sb", bufs=4
                            )
                        attnv_group(prev[0], prev[1], prev[2], qt, sub, o_sb[0])
                        if sub == 1:
                            attnv_finish_qt(prev[0], prev[1], qt, o_sb[0])
                    if kt > 0:
                        pump(per_kt)

            def attnv_tail(pair, cq, ets):
                for qt in range(NQT):
                    o_sb = epool.tile([P, 2, D], BF16, name="o_sb", tag="o_sb", bufs=4)
                    for sub in range(2):
                        attnv_group(pair, cq, ets, qt, sub, o_sb)
                        pump(2)
                    attnv_finish_qt(pair, cq, qt, o_sb)

            # ---- emission ------------------------------------------------
            # startup: chunk-0 k/q projections inline (first exp gates on
            # these only); everything else becomes prioritized fillers whose
            # emission is forced before any reader is emitted.
            run_units(proj_qk_units(wk, kts, 0, 0))
            run_units(proj_qk_units(wq, qts, 0, 0))

            k0u = {0: None}
            q0u = {0: None}
            vu = {}
            for cq in range(1, NQ):
                k0u[cq] = add_filler(proj_qk_units(wk, kts, 0, cq))
                for st in range(4 * (cq - 1), 4 * cq):
                    vu[st] = add_filler(proj_v_units(st))
                q0u[cq] = add_filler(proj_qk_units(wq, qts, 0, cq))
            for st in range(12, 16):
                vu[st] = add_filler(proj_v_units(st))
            k1u = {cq: add_filler(proj_qk_units(wk, kts, 1, cq)) for cq in range(NQ)}
            q1u = {cq: add_filler(proj_qk_units(wq, qts, 1, cq)) for cq in range(NQ)}

            PER_KT = {(0, 0): 3, (0, 1): 3}
            prev = None
            for pair in range(FT):
                ku, qu = (k0u, q0u) if pair == 0 else (k1u, q1u)
                for cq in range(NQ):
                    # emission-order guards: k chunk per kt-block, q chunk,
                    # and (for the deferred attn@v of the previous chunk)
                    # all v tiles.
                    if qu[cq] is not None:
                        qu[cq].finish()
                    if pair == 1:
                        for u in k1u.values():
                            u.finish()
                    if prev is not None:
                        for u in vu.values():
                            u.finish()
                        vu = {}
                    gate = (lambda kt: ku.get(kt // 4) if kt % 4 == 0 else None) if pair == 0 else None
                    ets = []
                    attn_phase1(pair, cq, PER_KT.get((pair, cq), 2), ets, prev=prev, kt_gate=gate)
                    prev = (pair, cq, ets)
            attnv_tail(*prev)
            while fillers:
                pump(64)

    nc.compile()
    return nc


_NC_CACHE = None


def _get_nc():
    global _NC_CACHE
    if _NC_CACHE is None:
        _NC_CACHE = _build()
    return _NC_CACHE


def _bf16(a: np.ndarray) -> np.ndarray:
    return np.ascontiguousarray(a, dtype=np.float32).astype(ml_dtypes.bfloat16)


def make_in_maps(x, Wq, Wk, Wv, Wo):
    in_maps = []
    xTs = [_bf16(x[b].T) for b in range(B)]
    for c in range(NCORES):
        b, hg = c // GPB, c % GPB
        fsl = slice(hg * FPC, (hg + 1) * FPC)
        in_maps.append({
            "xT": xTs[b],
            "wqT": _bf16(Wq[fsl, :].T),
            "wkT": _bf16(Wk[fsl, :].T),
            "wvT": _bf16(Wv[fsl, :].T),
            "woT": _bf16(Wo[:, fsl].T),
        })
    return in_maps


def kernel(x, Wq, bq, Wk, bk, Wv, bv, Wo, bo):
    x = np.asarray(x, dtype=np.float32)
    Wq, Wk, Wv, Wo = (np.asarray(a, dtype=np.float32) for a in (Wq, Wk, Wv, Wo))
    bq, bk, bv, bo = (np.asarray(a, dtype=np.float32) for a in (bq, bk, bv, bo))
    if np.any(bq) or np.any(bk) or np.any(bv):
        raise NotImplementedError("nonzero projection biases not supported")

    nc = _get_nc()
    in_maps = make_in_maps(x, Wq, Wk, Wv, Wo)
    res = run_bass_kernel_spmd(nc, in_maps, core_ids=list(range(NCORES)))
    out = np.empty((B, S, E), dtype=np.float32)
    for b in range(B):
        acc = res.results[b * GPB]["out"].astype(np.float32)
        for hg in range(1, GPB):
            acc = acc + res.results[b * GPB + hg]["out"].astype(np.float32)
        out[b] = acc
    out += bo[None, None, :]
    return out


# revision 26
# speedup vs baseline: 1.1300x; 1.0258x over previous
"""Trainium2 Bass kernel: multi-head attention (B=2, S=2048, E=1024, H=16).

Sharding: 8 cores = 2 batches x 4 head-groups. Core c handles batch c//4 and
heads [4*(c%4), 4*(c%4)+4) (256 feature columns of the projections).

v2 design (all-bf16, transposed attn@v):
  - inputs in bf16: xT [E,S], wqT/wkT/wvT [E,256], woT [256,E].
  - qT,kT [256,S] bf16 in [f,s] layout; v [S,256] bf16 in [s,f] layout with a
    ones column per head (col 64) that produces the softmax denominator.
  - scores^T tiles [128 kj, 512 qi] on PE (2 heads per exp tile), exp on ACT
    (sm_scale folded into the activation scale) -> et bf16.
  - attn@v TRANSPOSED: out [128 qi, 65] = et[kj,qi]^T-contract v[kj,65]; the
    65th column accumulates the denominator. Normalize fuses into the PSUM
    drain as a per-partition tensor_scalar multiply by 1/denom.
  - o_sb [128 qi, 128 f(2 heads)] is transposed to oT [f, s] via the DMA XBAR
    transpose (16x128 tiles), then out = oT^T @ wo per s-tile, out bf16 DMA.
  - host sums 4 partials per batch and adds bo.
"""

import numpy as np
import ml_dtypes

from collections import deque

import concourse.tile as tile
import concourse.mybir as mybir
from concourse import bacc
from concourse.bass_utils import run_bass_kernel_spmd

B, S, E, H, D = 2, 2048, 1024, 16, 64
NCORES = 8
GPB = NCORES // B      # head-groups (cores) per batch = 4
HPC = H // GPB         # heads per core = 4
FPC = HPC * D          # feature cols per core = 256
SM = float(D) ** -0.5  # softmax scale

BF16 = mybir.dt.bfloat16
F32 = mybir.dt.float32

P = 128
NE = E // P            # 8 e-tiles
NST = S // P           # 16 s-tiles (key tiles)
NQ = 4                 # qi chunks
QC = S // NQ           # 512
NQT = QC // P          # 4 qi-tiles per chunk
FT = FPC // P          # 2 f-tiles (head pairs) per core


def _build():
    nc = bacc.Bacc("TRN2", target_bir_lowering=False, debug=False)

    # weights arrive pre-tiled in SBUF layout (partition-major) so the DMAs
    # are single straight copies with 2KB+ contiguous runs
    xT_d = nc.dram_tensor("xT", [E, S], BF16, kind="ExternalInput")
    wq0_d = nc.dram_tensor("wq0", [P, NE, P], BF16, kind="ExternalInput")
    wq1_d = nc.dram_tensor("wq1", [P, NE, P], BF16, kind="ExternalInput")
    wk0_d = nc.dram_tensor("wk0", [P, NE, P], BF16, kind="ExternalInput")
    wk1_d = nc.dram_tensor("wk1", [P, NE, P], BF16, kind="ExternalInput")
    wv_d = nc.dram_tensor("wvT", [P, NE, FPC], BF16, kind="ExternalInput")
    wo_d = nc.dram_tensor("woT", [P, FT, E], BF16, kind="ExternalInput")
    out_d = nc.dram_tensor("out", [S, E], BF16, kind="ExternalOutput")

    with tile.TileContext(nc) as tc:
        with (
            tc.tile_pool(name="wpool", bufs=1) as wpool,
            tc.tile_pool(name="xpool", bufs=1) as xpool,
            tc.tile_pool(name="qkpool", bufs=1) as qkpool,
            tc.tile_pool(name="vpool", bufs=1) as vpool,
            tc.tile_pool(name="opool", bufs=1) as opool,
            tc.tile_pool(name="epool", bufs=3) as epool,
            tc.tile_pool(name="spool", bufs=2) as spool,
            tc.tile_pool(name="outpool", bufs=2) as outpool,
            tc.tile_pool(name="pspool", bufs=2, space="PSUM") as pspool,
            tc.tile_pool(name="popool", bufs=2, space="PSUM") as popool,
            tc.tile_pool(name="oaccpool", bufs=1, space="PSUM") as oaccpool,
        ):
            # ---- weights / x DMA (emission order = DMA queue order) -------
            wks = [wpool.tile([P, NE, P], BF16, name=f"wk{ft}") for ft in range(FT)]
            wqs = [wpool.tile([P, NE, P], BF16, name=f"wq{ft}") for ft in range(FT)]
            wv = wpool.tile([P, NE, FPC], BF16, name="wv")
            wo = wpool.tile([P, FT, E], BF16, name="wo")
            nc.sync.dma_start(out=wks[0], in_=wk0_d.ap())
            nc.sync.dma_start(out=wqs[0], in_=wq0_d.ap())

            xT_r = xT_d.ap().rearrange("(t p) s -> p t s", p=P)
            xt = xpool.tile([P, NE, S], BF16, name="xt")
            xts = [xt[:, et, :] for et in range(NE)]
            # chunk 0 in two pieces so projection matmuls stream behind
            nc.sync.dma_start(out=xt[:, 0:4, 0:QC], in_=xT_r[:, 0:4, 0:QC])
            nc.sync.dma_start(out=xt[:, 4:NE, 0:QC], in_=xT_r[:, 4:NE, 0:QC])
            nc.sync.dma_start(out=wv, in_=wv_d.ap())
            nc.sync.dma_start(out=wks[1], in_=wk1_d.ap())
            nc.sync.dma_start(out=wqs[1], in_=wq1_d.ap())
            # PE pstate warmup: junk matmuls on a memset tile (no DMA dep)
            # so the PE reaches full clock (3us continuous busy) before the
            # real projections start.
            junk_sb = wpool.tile([P, QC], BF16, name="junk_sb")
            nc.gpsimd.memset(junk_sb, 0.03125)
            for w in range(7):
                ps_junk = popool.tile([P, QC], F32, name="ps_junk", tag="po")
                nc.tensor.matmul(
                    ps_junk,
                    junk_sb[:, 0:P],
                    junk_sb,
                    start=True,
                    stop=True,
                )
            # chunks 1-3: one DMA per chunk
            for cq in range(1, NQ):
                csl = slice(cq * QC, (cq + 1) * QC)
                nc.sync.dma_start(out=xt[:, :, csl], in_=xT_r[:, :, csl])
            nc.sync.dma_start(out=wo, in_=wo_d.ap())

            # ---- SBUF working tiles --------------------------------------
            kts = [qkpool.tile([P, S], BF16, name=f"kt{ft}", tag=f"kt{ft}") for ft in range(FT)]
            qts = [qkpool.tile([P, S], BF16, name=f"qt{ft}", tag=f"qt{ft}") for ft in range(FT)]
            oTs = [opool.tile([P, S], BF16, name=f"oT{ft}", tag=f"oT{ft}") for ft in range(FT)]
            v_tiles = [
                vpool.tile([P, HPC, D + 1], BF16, name=f"v{st}", tag=f"v{st}")
                for st in range(NST)
            ]
            # ones column for the denominators (gpsimd memset, no DMA)
            for st in range(NST):
                nc.gpsimd.memset(v_tiles[st][:, :, D : D + 1], 1.0)

            # ---- filler machinery: generators emit one PE-side instruction
            # per next() call so independent matmul work lands inside the PE
            # idle gaps of the ACT-bound attention loop. Units can be force-
            # finished to guarantee writer-before-reader EMISSION order (the
            # tile framework only tracks deps on already-emitted writers).
            fillers = deque()

            class Unit:
                def __init__(self, gen):
                    self.gen = gen
                    self.done = False

                def step(self):
                    if self.done:
                        return False
                    try:
                        next(self.gen)
                        return True
                    except StopIteration:
                        self.done = True
                        return False

                def finish(self):
                    while not self.done:
                        self.step()
                    if fillers and fillers[0] is self:
                        fillers.popleft()

            def add_filler(gen, front=False):
                u = Unit(gen)
                if front:
                    fillers.appendleft(u)
                else:
                    fillers.append(u)
                return u

            def pump(n):
                for _ in range(n):
                    while fillers:
                        if fillers[0].step():
                            break
                        fillers.popleft()
                    else:
                        return

            # ---- projection groups ---------------------------------------
            def proj_qk_units(w_tiles, dst, ft, cq):
                ps = popool.tile([P, QC], F32, name="ps_qk", tag="po")
                csl = slice(cq * QC, (cq + 1) * QC)
                for et in range(NE):
                    nc.tensor.matmul(
                        ps,
                        w_tiles[ft][:, et, :],
                        xts[et][:, csl],
                        start=(et == 0),
                        stop=(et == NE - 1),
                    )
                    yield
                nc.vector.tensor_copy(dst[ft][:, csl], ps)
                yield

            def proj_v_units(st):
                vt = v_tiles[st]
                ps_v = popool.tile([P, FPC], F32, name="ps_v", tag="po")
                for et in range(NE):
                    nc.tensor.matmul(
                        ps_v,
                        xts[et][:, st * P : (st + 1) * P],
                        wv[:, et, :],
                        start=(et == 0),
                        stop=(et == NE - 1),
                    )
                    yield
                nc.vector.tensor_copy(
                    vt[:, :, 0:D], ps_v.rearrange("p (h d) -> p h d", d=D)
                )
                yield

            def run_units(gen):
                for _ in gen:
                    pass

            # ---- output projection (per s-tile, pipelined as filler) -----
            def outproj_units(st, tail=False):
                out_sb = outpool.tile([P, E], BF16, name="out_sb", tag="out_sb")
                for gc in range(2):
                    ps_out = popool.tile([P, QC], F32, name="ps_out", tag="po")
                    for ft in range(FT):
                        nc.tensor.matmul(
                            ps_out,
                            oTs[ft][:, st * P : (st + 1) * P],
                            wo[:, ft, gc * QC : (gc + 1) * QC],
                            start=(ft == 0),
                            stop=(ft == FT - 1),
                        )
                        yield
                    if tail:
                        # ACT is idle after the last exp
                        nc.scalar.activation(
                            out=out_sb[:, gc * QC : (gc + 1) * QC],
                            in_=ps_out,
                            func=mybir.ActivationFunctionType.Copy,
                        )
                    else:
                        nc.vector.tensor_copy(
                            out_sb[:, gc * QC : (gc + 1) * QC], ps_out
                        )
                    yield
                nc.sync.dma_start(
                    out=out_d.ap()[st * P : (st + 1) * P, :], in_=out_sb
                )

            # ---- attention -----------------------------------------------
            # Phase 1 per (pair, cq): 16 kt-steps of scores+exp; the 16 exp
            # tiles are retained in SBUF (double-buffered across chunks).
            # Phase 2 (pumped as filler during the NEXT chunk's phase 1):
            # per acc (qt, sub), 16 contiguous attn@v matmuls into one of two
            # ping-pong PSUM banks (HW allows only one open accumulation
            # group per bank), then recip+normalize drain and, per qt, an
            # XBAR transpose into oT.
            oaccs = [
                oaccpool.tile([P, D + 1], F32, name=f"oacc{i}", tag=f"oacc{i}")
                for i in range(2)
            ]

            gi = [0]  # global attn@v group parity for psum ping-pong
            # outproj is deferred ~2 qt-tiles behind its XBAR transpose so
            # the transpose's DMA-pipeline latency (~3us) never blocks the
            # in-order PE stream.
            pending_outproj = deque()

            def attnv_group(pair, cq, ets, qt, sub, o_sb, tail=False):
                """One contiguous attn@v accumulation group (16 matmuls) plus
                its reciprocal + normalize drain. HW allows only one open
                accumulation group per PSUM bank, hence contiguous + two
                ping-pong banks."""
                acc = oaccs[gi[0] % 2]
                gi[0] += 1
                for kt in range(NST):
                    nc.tensor.matmul(
                        acc,
                        ets[kt][:, sub, qt * P : (qt + 1) * P],
                        v_tiles[kt][:, 2 * pair + sub, :],
                        start=(kt == 0),
                        stop=(kt == NST - 1),
                    )
                r = spool.tile([P, 1], F32, name="r", tag="r", bufs=4)
                nc.vector.reciprocal(r, acc[:, D : D + 1])
                if tail:
                    nc.scalar.activation(
                        out=o_sb[:, sub, :],
                        in_=acc[:, 0:D],
                        func=mybir.ActivationFunctionType.Copy,
                        scale=r,
                    )
                else:
                    nc.vector.tensor_scalar(
                        out=o_sb[:, sub, :],
                        in0=acc[:, 0:D],
                        scalar1=r,
                        scalar2=None,
                        op0=mybir.AluOpType.mult,
                    )

            def attnv_finish_qt(pair, cq, qt, o_sb, tail=False):
                nc.sync.dma_start_transpose(
                    out=oTs[pair][:, cq * QC + qt * P : cq * QC + (qt + 1) * P],
                    in_=o_sb,
                )
                if pair == 1:
                    pending_outproj.append((cq * NQT + qt, tail))
                    if len(pending_outproj) > 2:
                        add_filler(outproj_units(*pending_outproj.popleft()))

            def attn_phase1(pair, cq, per_kt, ets, specs=(), forces=None):
                """Scores+exp for (pair, cq); deferred attn@v groups (specs,
                from chunks >= 1 behind) are interleaved deterministically so
                every engine stream is emitted in a feasible execution
                order. `forces` maps kt -> filler Units that must be fully
                emitted before that kt's scores (k/q chunk guards and
                prefetches)."""
                csl = slice(cq * QC, (cq + 1) * QC)
                o_sb = [None]
                # spread the groups over the 16 kt-steps
                spec_at = {}
                n = len(specs)
                if n <= 8:
                    slots = [2 * i + 1 for i in range(n)]
                elif n <= 12:
                    slots = [1, 2, 3, 5, 6, 7, 9, 10, 11, 13, 14, 15][:n]
                else:
                    slots = list(range(n))
                for i, s in enumerate(specs):
                    spec_at[slots[i]] = s
                for kt in range(NST):
                    for u in (forces or {}).get(kt, []):
                        if u is not None:
                            u.finish()
                    ps_s = pspool.tile([P, 2, QC], F32, name="ps_s", tag="ps_s")
                    et_t = epool.tile([P, 2, QC], BF16, name=f"et{kt}", tag=f"et{kt}", bufs=3)
                    ets.append(et_t)
                    for sub in range(2):
                        lo, hi = sub * D, (sub + 1) * D
                        nc.tensor.matmul(
                            ps_s[:, sub, :],
                            kts[pair][lo:hi, kt * P : (kt + 1) * P],
                            qts[pair][lo:hi, csl],
                            start=True,
                            stop=True,
                        )
                    nc.scalar.activation(
                        out=et_t,
                        in_=ps_s,
                        func=mybir.ActivationFunctionType.Exp,
                        scale=SM,
                    )
                    if kt in spec_at:
                        gp, gcq, gets, qt, sub = spec_at[kt]
                        if sub == 0:
                            o_sb[0] = epool.tile(
                                [P, 2, D], BF16, name="o_sb", tag="o_sb", bufs=4
                            )
                        attnv_group(gp, gcq, gets, qt, sub, o_sb[0])
                        if sub == 1:
                            attnv_finish_qt(gp, gcq, qt, o_sb[0])
                    if kt > 0:
                        pump(per_kt)

            def attnv_tail(pair, cq, ets):
                while pending_outproj:
                    add_filler(outproj_units(*pending_outproj.popleft()))
                for qt in range(NQT):
                    o_sb = epool.tile([P, 2, D], BF16, name="o_sb", tag="o_sb", bufs=4)
                    for sub in range(2):
                        attnv_group(pair, cq, ets, qt, sub, o_sb, tail=True)
                        pump(4)
                    attnv_finish_qt(pair, cq, qt, o_sb, tail=True)
                    pump(4)
                while pending_outproj:
                    run_units(outproj_units(*pending_outproj.popleft()))


            # ---- emission ------------------------------------------------
            # startup: chunk-0 k/q projections inline (first exp gates on
            # these only); everything else becomes prioritized fillers whose
            # emission is forced before any reader is emitted.
            run_units(proj_qk_units(wks, kts, 0, 0))
            run_units(proj_qk_units(wqs, qts, 0, 0))

            k0u = {0: None}
            q0u = {0: None}
            vu = {}
            for cq in range(1, NQ):
                k0u[cq] = add_filler(proj_qk_units(wks, kts, 0, cq))
                for st in range(4 * (cq - 1), 4 * cq):
                    vu[st] = add_filler(proj_v_units(st))
                q0u[cq] = add_filler(proj_qk_units(wqs, qts, 0, cq))
            for st in range(12, 16):
                vu[st] = add_filler(proj_v_units(st))
            k1u = {cq: add_filler(proj_qk_units(wks, kts, 1, cq)) for cq in range(NQ)}
            q1u = {cq: add_filler(proj_qk_units(wqs, qts, 1, cq)) for cq in range(NQ)}

            # chunk schedule with deferred attn@v: 2-chunk lag at the start
            # (the v projections can't fit into the first two PE windows),
            # catch-up in (1,0), single-chunk tail.
            schedule = [(0, 0), (0, 1), (0, 2), (0, 3), (1, 0), (1, 1), (1, 2), (1, 3)]
            groups_in = {
                (0, 2): [((0, 0), 0, 8)],
                (0, 3): [((0, 1), 0, 8)],
                (1, 0): [((0, 2), 0, 8), ((0, 3), 0, 4)],
                (1, 1): [((0, 3), 4, 8), ((1, 0), 0, 8)],
                (1, 2): [((1, 1), 0, 8)],
                (1, 3): [((1, 2), 0, 8)],
            }
            PER_KT = {(0, 0): 2, (0, 1): 4}
            chunk_ets = {}
            v_forced = False
            for ci, (pair, cq) in enumerate(schedule):
                ku, qu = (k0u, q0u) if pair == 0 else (k1u, q1u)
                qu.get(cq) and qu[cq].finish()
                specs = []
                for gkey, g0, g1 in groups_in.get((pair, cq), []):
                    if not v_forced:
                        for u in vu.values():
                            u.finish()
                        v_forced = True
                    gets = chunk_ets[gkey]
                    for g in range(g0, g1):
                        specs.append((gkey[0], gkey[1], gets, g // 2, g % 2))
                forces = {kt: [ku.get(kt // 4)] for kt in (0, 4, 8, 12)}
                if ci + 1 < len(schedule):
                    np_pair, np_cq = schedule[ci + 1]
                    nqu = q0u if np_pair == 0 else q1u
                    forces.setdefault(6, []).append(nqu.get(np_cq))
                if (pair, cq) == (0, 3):
                    for i in range(4):
                        forces.setdefault(2 + 4 * i, []).append(k1u[i])
                ets = []
                attn_phase1(pair, cq, PER_KT.get((pair, cq), 2), ets, specs, forces)
                chunk_ets[(pair, cq)] = ets
            attnv_tail(1, 3, chunk_ets[(1, 3)])
            while fillers:
                pump(64)

    nc.compile()
    return nc


_NC_CACHE = None


def _get_nc():
    global _NC_CACHE
    if _NC_CACHE is None:
        _NC_CACHE = _build()
    return _NC_CACHE


def _bf16(a: np.ndarray) -> np.ndarray:
    return np.ascontiguousarray(a, dtype=np.float32).astype(ml_dtypes.bfloat16)


def _tile_w(W, fsl):
    """[E, FPC] -> SBUF layout [P, NE, FPC] (partition-major, contiguous)."""
    wt = np.ascontiguousarray(W[fsl, :].T)  # [E, FPC]
    return np.ascontiguousarray(wt.reshape(NE, P, FPC).transpose(1, 0, 2))


def make_in_maps(x, Wq, Wk, Wv, Wo):
    in_maps = []
    xTs = [_bf16(x[b].T) for b in range(B)]
    for c in range(NCORES):
        b, hg = c // GPB, c % GPB
        fsl = slice(hg * FPC, (hg + 1) * FPC)
        wq_t = _bf16(_tile_w(Wq, fsl))
        wk_t = _bf16(_tile_w(Wk, fsl))
        wo_t = np.ascontiguousarray(Wo[:, fsl].T)  # [FPC, E]
        wo_t = _bf16(np.ascontiguousarray(wo_t.reshape(FT, P, E).transpose(1, 0, 2)))
        in_maps.append({
            "xT": xTs[b],
            "wq0": np.ascontiguousarray(wq_t[:, :, 0:P]),
            "wq1": np.ascontiguousarray(wq_t[:, :, P:FPC]),
            "wk0": np.ascontiguousarray(wk_t[:, :, 0:P]),
            "wk1": np.ascontiguousarray(wk_t[:, :, P:FPC]),
            "wvT": _bf16(_tile_w(Wv, fsl)),
            "woT": wo_t,
        })
    return in_maps


def kernel(x, Wq, bq, Wk, bk, Wv, bv, Wo, bo):
    x = np.asarray(x, dtype=np.float32)
    Wq, Wk, Wv, Wo = (np.asarray(a, dtype=np.float32) for a in (Wq, Wk, Wv, Wo))
    bq, bk, bv, bo = (np.asarray(a, dtype=np.float32) for a in (bq, bk, bv, bo))
    if np.any(bq) or np.any(bk) or np.any(bv):
        raise NotImplementedError("nonzero projection biases not supported")

    nc = _get_nc()
    in_maps = make_in_maps(x, Wq, Wk, Wv, Wo)
    res = run_bass_kernel_spmd(nc, in_maps, core_ids=list(range(NCORES)))
    out = np.empty((B, S, E), dtype=np.float32)
    for b in range(B):
        acc = res.results[b * GPB]["out"].astype(np.float32)
        for hg in range(1, GPB):
            acc = acc + res.results[b * GPB + hg]["out"].astype(np.float32)
        out[b] = acc
    out += bo[None, None, :]
    return out


# revision 32
# speedup vs baseline: 1.1866x; 1.0501x over previous
"""Trainium2 Bass kernel: multi-head attention (B=2, S=2048, E=1024, H=16).

Sharding: 8 cores = 2 batches x 4 head-groups. Core c handles batch c//4 and
heads [4*(c%4), 4*(c%4)+4) (256 feature columns of the projections).

v2 design (all-bf16, transposed attn@v):
  - inputs in bf16: xT [E,S], wqT/wkT/wvT [E,256], woT [256,E].
  - qT,kT [256,S] bf16 in [f,s] layout; v [S,256] bf16 in [s,f] layout with a
    ones column per head (col 64) that produces the softmax denominator.
  - scores^T tiles [128 kj, 512 qi] on PE (2 heads per exp tile), exp on ACT
    (sm_scale folded into the activation scale) -> et bf16.
  - attn@v TRANSPOSED: out [128 qi, 65] = et[kj,qi]^T-contract v[kj,65]; the
    65th column accumulates the denominator. Normalize fuses into the PSUM
    drain as a per-partition tensor_scalar multiply by 1/denom.
  - o_sb [128 qi, 128 f(2 heads)] is transposed to oT [f, s] via the DMA XBAR
    transpose (16x128 tiles), then out = oT^T @ wo per s-tile, out bf16 DMA.
  - host sums 4 partials per batch and adds bo.
"""

import numpy as np
import ml_dtypes

from collections import deque

import concourse.tile as tile
import concourse.mybir as mybir
from concourse import bacc
from concourse.bass_utils import run_bass_kernel_spmd

B, S, E, H, D = 2, 2048, 1024, 16, 64
NCORES = 8
GPB = NCORES // B      # head-groups (cores) per batch = 4
HPC = H // GPB         # heads per core = 4
FPC = HPC * D          # feature cols per core = 256
SM = float(D) ** -0.5  # softmax scale

BF16 = mybir.dt.bfloat16
F32 = mybir.dt.float32

P = 128
NE = E // P            # 8 e-tiles
NST = S // P           # 16 s-tiles (key tiles)
NQ = 4                 # qi chunks
QC = S // NQ           # 512
NQT = QC // P          # 4 qi-tiles per chunk
FT = FPC // P          # 2 f-tiles (head pairs) per core


def _build():
    nc = bacc.Bacc("TRN2", target_bir_lowering=False, debug=False)

    # weights arrive pre-tiled in SBUF layout (partition-major) so the DMAs
    # are single straight copies with 2KB+ contiguous runs
    xT_d = nc.dram_tensor("xT", [E, S], BF16, kind="ExternalInput")
    wq0_d = nc.dram_tensor("wq0", [P, NE, P], BF16, kind="ExternalInput")
    wq1_d = nc.dram_tensor("wq1", [P, NE, P], BF16, kind="ExternalInput")
    wk0_d = nc.dram_tensor("wk0", [P, NE, P], BF16, kind="ExternalInput")
    wk1_d = nc.dram_tensor("wk1", [P, NE, P], BF16, kind="ExternalInput")
    wv_d = nc.dram_tensor("wvT", [P, NE, FPC], BF16, kind="ExternalInput")
    wo_d = nc.dram_tensor("woT", [P, FT, E], BF16, kind="ExternalInput")
    out_d = nc.dram_tensor("out", [S, E], BF16, kind="ExternalOutput")

    with tile.TileContext(nc) as tc:
        with (
            tc.tile_pool(name="wpool", bufs=1) as wpool,
            tc.tile_pool(name="xpool", bufs=1) as xpool,
            tc.tile_pool(name="qkpool", bufs=1) as qkpool,
            tc.tile_pool(name="vpool", bufs=1) as vpool,
            tc.tile_pool(name="opool", bufs=1) as opool,
            tc.tile_pool(name="epool", bufs=3) as epool,
            tc.tile_pool(name="spool", bufs=2) as spool,
            tc.tile_pool(name="outpool", bufs=4) as outpool,
            tc.tile_pool(name="pspool", bufs=2, space="PSUM") as pspool,
            tc.tile_pool(name="popool", bufs=2, space="PSUM") as popool,
            tc.tile_pool(name="oaccpool", bufs=1, space="PSUM") as oaccpool,
        ):
            # ---- weights / x DMA (emission order = DMA queue order) -------
            wks = [wpool.tile([P, NE, P], BF16, name=f"wk{ft}") for ft in range(FT)]
            wqs = [wpool.tile([P, NE, P], BF16, name=f"wq{ft}") for ft in range(FT)]
            wv = wpool.tile([P, NE, FPC], BF16, name="wv")
            wo = wpool.tile([P, FT, E], BF16, name="wo")
            nc.sync.dma_start(out=wks[0], in_=wk0_d.ap())
            nc.sync.dma_start(out=wqs[0], in_=wq0_d.ap())

            xT_r = xT_d.ap().rearrange("(t p) s -> p t s", p=P)
            xt = xpool.tile([P, NE, S], BF16, name="xt")
            xts = [xt[:, et, :] for et in range(NE)]
            # chunk 0 in two pieces so projection matmuls stream behind
            nc.sync.dma_start(out=xt[:, 0:4, 0:QC], in_=xT_r[:, 0:4, 0:QC])
            nc.sync.dma_start(out=xt[:, 4:NE, 0:QC], in_=xT_r[:, 4:NE, 0:QC])
            nc.sync.dma_start(out=wv, in_=wv_d.ap())
            nc.sync.dma_start(out=wks[1], in_=wk1_d.ap())
            nc.sync.dma_start(out=wqs[1], in_=wq1_d.ap())
            # PE pstate warmup: junk matmuls on a memset tile (no DMA dep)
            # so the PE reaches full clock (3us continuous busy) before the
            # real projections start.
            junk_sb = wpool.tile([P, QC], BF16, name="junk_sb")
            nc.gpsimd.memset(junk_sb, 0.03125)
            for w in range(7):
                ps_junk = popool.tile([P, QC], F32, name="ps_junk", tag="po")
                nc.tensor.matmul(
                    ps_junk,
                    junk_sb[:, 0:P],
                    junk_sb,
                    start=True,
                    stop=True,
                )
            # chunks 1-3: one DMA per chunk
            for cq in range(1, NQ):
                csl = slice(cq * QC, (cq + 1) * QC)
                nc.sync.dma_start(out=xt[:, :, csl], in_=xT_r[:, :, csl])
            nc.sync.dma_start(out=wo, in_=wo_d.ap())

            # ---- SBUF working tiles --------------------------------------
            kts = [qkpool.tile([P, S], BF16, name=f"kt{ft}", tag=f"kt{ft}") for ft in range(FT)]
            qts = [qkpool.tile([P, S], BF16, name=f"qt{ft}", tag=f"qt{ft}") for ft in range(FT)]
            oTs = [opool.tile([P, S], BF16, name=f"oT{ft}", tag=f"oT{ft}") for ft in range(FT)]
            v_tiles = [
                vpool.tile([P, HPC, D + 1], BF16, name=f"v{st}", tag=f"v{st}")
                for st in range(NST)
            ]
            # ones column for the denominators (gpsimd memset, no DMA)
            for st in range(NST):
                nc.gpsimd.memset(v_tiles[st][:, :, D : D + 1], 1.0)

            # ---- filler machinery: generators emit one PE-side instruction
            # per next() call so independent matmul work lands inside the PE
            # idle gaps of the ACT-bound attention loop. Units can be force-
            # finished to guarantee writer-before-reader EMISSION order (the
            # tile framework only tracks deps on already-emitted writers).
            fillers = deque()

            class Unit:
                def __init__(self, gen):
                    self.gen = gen
                    self.done = False

                def step(self):
                    if self.done:
                        return False
                    try:
                        next(self.gen)
                        return True
                    except StopIteration:
                        self.done = True
                        return False

                def finish(self):
                    while not self.done:
                        self.step()
                    if fillers and fillers[0] is self:
                        fillers.popleft()

            def add_filler(gen, front=False):
                u = Unit(gen)
                if front:
                    fillers.appendleft(u)
                else:
                    fillers.append(u)
                return u

            def pump(n):
                for _ in range(n):
                    while fillers:
                        if fillers[0].step():
                            break
                        fillers.popleft()
                    else:
                        return

            # ---- projection groups ---------------------------------------
            def proj_qk_units(w_tiles, dst, ft, cq):
                ps = popool.tile([P, QC], F32, name="ps_qk", tag="po")
                csl = slice(cq * QC, (cq + 1) * QC)
                for et in range(NE):
                    nc.tensor.matmul(
                        ps,
                        w_tiles[ft][:, et, :],
                        xts[et][:, csl],
                        start=(et == 0),
                        stop=(et == NE - 1),
                    )
                    yield
                nc.vector.tensor_copy(dst[ft][:, csl], ps)
                yield

            def proj_v_units(st):
                vt = v_tiles[st]
                ps_v = popool.tile([P, FPC], F32, name="ps_v", tag="po")
                for et in range(NE):
                    nc.tensor.matmul(
                        ps_v,
                        xts[et][:, st * P : (st + 1) * P],
                        wv[:, et, :],
                        start=(et == 0),
                        stop=(et == NE - 1),
                    )
                    yield
                nc.vector.tensor_copy(
                    vt[:, :, 0:D], ps_v.rearrange("p (h d) -> p h d", d=D)
                )
                yield

            def run_units(gen):
                for _ in gen:
                    pass

            # ---- output projection (per s-tile, pipelined as filler) -----
            def outproj_units(st, tail=False):
                out_sb = outpool.tile([P, E], BF16, name="out_sb", tag="out_sb")
                for gc in range(2):
                    ps_out = popool.tile([P, QC], F32, name="ps_out", tag="po")
                    for ft in range(FT):
                        nc.tensor.matmul(
                            ps_out,
                            oTs[ft][:, st * P : (st + 1) * P],
                            wo[:, ft, gc * QC : (gc + 1) * QC],
                            start=(ft == 0),
                            stop=(ft == FT - 1),
                        )
                        yield
                    if tail:
                        # ACT is idle after the last exp; per-gc DMA on the
                        # ACT hwdge queue shortens the final chain and avoids
                        # SP head-of-line blocking behind the transposes.
                        nc.scalar.activation(
                            out=out_sb[:, gc * QC : (gc + 1) * QC],
                            in_=ps_out,
                            func=mybir.ActivationFunctionType.Copy,
                        )
                        nc.scalar.dma_start(
                            out=out_d.ap()[
                                st * P : (st + 1) * P, gc * QC : (gc + 1) * QC
                            ],
                            in_=out_sb[:, gc * QC : (gc + 1) * QC],
                        )
                    else:
                        nc.vector.tensor_copy(
                            out_sb[:, gc * QC : (gc + 1) * QC], ps_out
                        )
                    yield
                if not tail:
                    nc.sync.dma_start(
                        out=out_d.ap()[st * P : (st + 1) * P, :], in_=out_sb
                    )

            # ---- attention -----------------------------------------------
            # Phase 1 per (pair, cq): 16 kt-steps of scores+exp; the 16 exp
            # tiles are retained in SBUF (double-buffered across chunks).
            # Phase 2 (pumped as filler during the NEXT chunk's phase 1):
            # per acc (qt, sub), 16 contiguous attn@v matmuls into one of two
            # ping-pong PSUM banks (HW allows only one open accumulation
            # group per bank), then recip+normalize drain and, per qt, an
            # XBAR transpose into oT.
            oaccs = [
                oaccpool.tile([P, D + 1], F32, name=f"oacc{i}", tag=f"oacc{i}")
                for i in range(2)
            ]

            gi = [0]  # global attn@v group parity for psum ping-pong
            # outproj is deferred ~2 qt-tiles behind its XBAR transpose so
            # the transpose's DMA-pipeline latency (~3us) never blocks the
            # in-order PE stream.
            pending_outproj = deque()

            def attnv_group(pair, cq, ets, qt, sub, o_sb, tail=False):
                """One contiguous attn@v accumulation group (16 matmuls) plus
                its reciprocal + normalize drain. HW allows only one open
                accumulation group per PSUM bank, hence contiguous + two
                ping-pong banks."""
                acc = oaccs[gi[0] % 2]
                gi[0] += 1
                for kt in range(NST):
                    nc.tensor.matmul(
                        acc,
                        ets[kt][:, sub, qt * P : (qt + 1) * P],
                        v_tiles[kt][:, 2 * pair + sub, :],
                        start=(kt == 0),
                        stop=(kt == NST - 1),
                    )
                r = spool.tile([P, 1], F32, name="r", tag="r", bufs=4)
                nc.vector.reciprocal(r, acc[:, D : D + 1])
                nc.vector.tensor_scalar(
                    out=o_sb[:, sub, :],
                    in0=acc[:, 0:D],
                    scalar1=r,
                    scalar2=None,
                    op0=mybir.AluOpType.mult,
                )

            def attnv_finish_qt(pair, cq, qt, o_sb, tail=False):
                nc.sync.dma_start_transpose(
                    out=oTs[pair][:, cq * QC + qt * P : cq * QC + (qt + 1) * P],
                    in_=o_sb,
                )
                if pair == 1:
                    pending_outproj.append((cq * NQT + qt, tail))
                    if len(pending_outproj) > 2:
                        add_filler(outproj_units(*pending_outproj.popleft()))

            def attn_phase1(pair, cq, per_kt, ets, specs=(), forces=None):
                """Scores+exp for (pair, cq); deferred attn@v groups (specs,
                from chunks >= 1 behind) are interleaved deterministically so
                every engine stream is emitted in a feasible execution
                order. `forces` maps kt -> filler Units that must be fully
                emitted before that kt's scores (k/q chunk guards and
                prefetches)."""
                csl = slice(cq * QC, (cq + 1) * QC)
                o_sb = [None]
                # spread the groups over the 16 kt-steps
                spec_at = {}
                n = len(specs)
                if n <= 8:
                    slots = [2 * i + 1 for i in range(n)]
                elif n <= 12:
                    slots = [1, 2, 3, 5, 6, 7, 9, 10, 11, 13, 14, 15][:n]
                else:
                    slots = list(range(n))
                for i, s in enumerate(specs):
                    spec_at[slots[i]] = s
                for kt in range(NST):
                    for u in (forces or {}).get(kt, []):
                        if u is not None:
                            u.finish()
                    ps_s = pspool.tile([P, 2, QC], F32, name="ps_s", tag="ps_s")
                    et_t = epool.tile([P, 2, QC], BF16, name=f"et{kt}", tag=f"et{kt}", bufs=3)
                    ets.append(et_t)
                    for sub in range(2):
                        lo, hi = sub * D, (sub + 1) * D
                        nc.tensor.matmul(
                            ps_s[:, sub, :],
                            kts[pair][lo:hi, kt * P : (kt + 1) * P],
                            qts[pair][lo:hi, csl],
                            start=True,
                            stop=True,
                        )
                    nc.scalar.activation(
                        out=et_t,
                        in_=ps_s,
                        func=mybir.ActivationFunctionType.Exp,
                        scale=SM,
                    )
                    if kt in spec_at:
                        gp, gcq, gets, qt, sub = spec_at[kt]
                        if sub == 0:
                            o_sb[0] = epool.tile(
                                [P, 2, D], BF16, name="o_sb", tag="o_sb", bufs=4
                            )
                        attnv_group(gp, gcq, gets, qt, sub, o_sb[0])
                        if sub == 1:
                            attnv_finish_qt(gp, gcq, qt, o_sb[0])
                    if kt > 0:
                        pump(per_kt)

            def attnv_tail(pair, cq, ets):
                while pending_outproj:
                    add_filler(outproj_units(*pending_outproj.popleft()))
                # all groups + drains + transposes first; the tail outprojs
                # run strictly after so the last transpose is never queued
                # behind an out DMA.
                o_sbs = []
                for qt in range(NQT):
                    o_sb = epool.tile([P, 2, D], BF16, name="o_sb", tag="o_sb", bufs=4)
                    o_sbs.append(o_sb)
                    for sub in range(2):
                        attnv_group(pair, cq, ets, qt, sub, o_sb, tail=True)
                        pump(3)
                for qt in range(NQT):
                    nc.sync.dma_start_transpose(
                        out=oTs[pair][:, cq * QC + qt * P : cq * QC + (qt + 1) * P],
                        in_=o_sbs[qt],
                    )
                while pending_outproj:
                    run_units(outproj_units(*pending_outproj.popleft()))
                for st in range(cq * NQT, (cq + 1) * NQT):
                    run_units(outproj_units(st))


            # ---- emission ------------------------------------------------
            # startup: chunk-0 k/q projections inline (first exp gates on
            # these only); everything else becomes prioritized fillers whose
            # emission is forced before any reader is emitted.
            run_units(proj_qk_units(wks, kts, 0, 0))
            run_units(proj_qk_units(wqs, qts, 0, 0))

            k0u = {0: None}
            q0u = {0: None}
            vu = {}
            for cq in range(1, NQ):
                k0u[cq] = add_filler(proj_qk_units(wks, kts, 0, cq))
                for st in range(4 * (cq - 1), 4 * cq):
                    vu[st] = add_filler(proj_v_units(st))
                q0u[cq] = add_filler(proj_qk_units(wqs, qts, 0, cq))
            for st in range(12, 16):
                vu[st] = add_filler(proj_v_units(st))
            k1u = {cq: add_filler(proj_qk_units(wks, kts, 1, cq)) for cq in range(NQ)}
            q1u = {cq: add_filler(proj_qk_units(wqs, qts, 1, cq)) for cq in range(NQ)}

            # chunk schedule with deferred attn@v: 2-chunk lag at the start
            # (the v projections can't fit into the first two PE windows),
            # catch-up in (1,0), single-chunk tail.
            schedule = [(0, 0), (0, 1), (0, 2), (0, 3), (1, 0), (1, 1), (1, 2), (1, 3)]
            groups_in = {
                (0, 2): [((0, 0), 0, 8)],
                (0, 3): [((0, 1), 0, 8)],
                (1, 0): [((0, 2), 0, 8), ((0, 3), 0, 4)],
                (1, 1): [((0, 3), 4, 8), ((1, 0), 0, 8)],
                (1, 2): [((1, 1), 0, 8)],
                (1, 3): [((1, 2), 0, 8)],
            }
            PER_KT = {(0, 0): 2, (0, 1): 4}
            chunk_ets = {}
            v_forced = False
            for ci, (pair, cq) in enumerate(schedule):
                ku, qu = (k0u, q0u) if pair == 0 else (k1u, q1u)
                qu.get(cq) and qu[cq].finish()
                specs = []
                for gkey, g0, g1 in groups_in.get((pair, cq), []):
                    if not v_forced:
                        for u in vu.values():
                            u.finish()
                        v_forced = True
                    gets = chunk_ets[gkey]
                    for g in range(g0, g1):
                        specs.append((gkey[0], gkey[1], gets, g // 2, g % 2))
                forces = {kt: [ku.get(kt // 4)] for kt in (0, 4, 8, 12)}
                if ci + 1 < len(schedule):
                    np_pair, np_cq = schedule[ci + 1]
                    nqu = q0u if np_pair == 0 else q1u
                    forces.setdefault(6, []).append(nqu.get(np_cq))
                if (pair, cq) == (0, 3):
                    for i in range(4):
                        forces.setdefault(2 + 4 * i, []).append(k1u[i])
                ets = []
                attn_phase1(pair, cq, PER_KT.get((pair, cq), 2), ets, specs, forces)
                chunk_ets[(pair, cq)] = ets
            attnv_tail(1, 3, chunk_ets[(1, 3)])
            while fillers:
                pump(64)

    nc.compile()
    return nc


_NC_CACHE = None


def _get_nc():
    global _NC_CACHE
    if _NC_CACHE is None:
        _NC_CACHE = _build()
    return _NC_CACHE


def _bf16(a: np.ndarray) -> np.ndarray:
    return np.ascontiguousarray(a, dtype=np.float32).astype(ml_dtypes.bfloat16)


def _tile_w(W, fsl):
    """[E, FPC] -> SBUF layout [P, NE, FPC] (partition-major, contiguous)."""
    wt = np.ascontiguousarray(W[fsl, :].T)  # [E, FPC]
    return np.ascontiguousarray(wt.reshape(NE, P, FPC).transpose(1, 0, 2))


def make_in_maps(x, Wq, Wk, Wv, Wo):
    in_maps = []
    xTs = [_bf16(x[b].T) for b in range(B)]
    for c in range(NCORES):
        b, hg = c // GPB, c % GPB
        fsl = slice(hg * FPC, (hg + 1) * FPC)
        wq_t = _bf16(_tile_w(Wq, fsl))
        wk_t = _bf16(_tile_w(Wk, fsl))
        wo_t = np.ascontiguousarray(Wo[:, fsl].T)  # [FPC, E]
        wo_t = _bf16(np.ascontiguousarray(wo_t.reshape(FT, P, E).transpose(1, 0, 2)))
        in_maps.append({
            "xT": xTs[b],
            "wq0": np.ascontiguousarray(wq_t[:, :, 0:P]),
            "wq1": np.ascontiguousarray(wq_t[:, :, P:FPC]),
            "wk0": np.ascontiguousarray(wk_t[:, :, 0:P]),
            "wk1": np.ascontiguousarray(wk_t[:, :, P:FPC]),
            "wvT": _bf16(_tile_w(Wv, fsl)),
            "woT": wo_t,
        })
    return in_maps


def kernel(x, Wq, bq, Wk, bk, Wv, bv, Wo, bo):
    x = np.asarray(x, dtype=np.float32)
    Wq, Wk, Wv, Wo = (np.asarray(a, dtype=np.float32) for a in (Wq, Wk, Wv, Wo))
    bq, bk, bv, bo = (np.asarray(a, dtype=np.float32) for a in (bq, bk, bv, bo))
    if np.any(bq) or np.any(bk) or np.any(bv):
        raise NotImplementedError("nonzero projection biases not supported")

    nc = _get_nc()
    in_maps = make_in_maps(x, Wq, Wk, Wv, Wo)
    res = run_bass_kernel_spmd(nc, in_maps, core_ids=list(range(NCORES)))
    out = np.empty((B, S, E), dtype=np.float32)
    for b in range(B):
        acc = res.results[b * GPB]["out"].astype(np.float32)
        for hg in range(1, GPB):
            acc = acc + res.results[b * GPB + hg]["out"].astype(np.float32)
        out[b] = acc
    out += bo[None, None, :]
    return out
